# revision 26
# baseline (speedup 1.0000x reference)
"""BiAffine layer kernel for 8 Trainium2 NeuronCores.

Reference computation (per batch b):
  s = relu(x @ sW.T + sb)                  [L, E]
  t = relu(x @ tW.T + tb)                  [L, E]
  key = (s @ blW.T).reshape(L, E, N)
  out1[i, n, l] = sum_e key[i, e, n] * t[l, e]
  su = s @ Wu.T + f2b ; tv = t @ Wv.T      (Wu, Wv = f2W[:, :E], f2W[:, E:])
  h[i, j, :] = relu(su[i] + tv[j])
  out2[i, n, j] = sum_e h[i, j, e] * f3W[n, e] + f3b[n]
  out = out1 + out2                        [L, N, L]

Sharding: 8 cores = 2 batches x 4 blocks of 128 source positions (i).

PSUM layout: one pair-tile [128, 1024] (2 banks) holds TWO octets of 8 i's
each; within a 512-col half, rows 32k + 12s + n hold (i = 8o + 2k + s, n),
8 pad rows per 32-group carry garbage that the output DMA skips.

h production per (i, ec) chunk [128, 512] fp16, true h = relu(tv + su):
  i%8 in 0..5 -> DVE tensor_scalar (in0=tv fp16, scalar1=su[:, i] fp32,
    op0=add, op1=max 0.0) - sustains ~263ns/op.
  i%8 in 6,7 -> ACT activation(Relu, bias=su[:, i]) ~613ns/op.
No correction terms needed; flush bias is just f3b (row-mapped).

key is produced PRE-PACKED via strided matmul PSUM out APs (cols
32d + 12s + n in a [128, 2048] 4-bank tile per ec), then 4 contiguous
[128, 512] casts to SBUF fp16. out1 = 8 M=32 matmuls per octet from the
packed key; out2 = 16 M=24 matmuls vs zero-block-padded f3 stationaries;
4-way PE column-group concurrency throughout.

All prep matmuls run in fp16 (x, sW, tW shipped fp16).  The first DEFER
octets emit h/out2 only; their out1 matmuls (which need key) are emitted
after octet DEFER-1 so the PE is not head-of-line blocked on key
production.  Flush: one ACT [128, 1024] copy+f3b per pair, one output DMA
per pair (2-level partition AP).
"""

import sys

sys.path.insert(0, "/opt/trn_rl_repo")

import numpy as np

B, L, H, E, N = 2, 512, 768, 256, 12
EC = E // 128  # 2 e-chunks
HC = H // 128  # 6 h-chunks
IB = L // 4  # 128 i's per core
NCORES = 8
OCTS = IB // 8  # 16
PAIRS = OCTS // 2  # 8
DEFER = 4  # octets whose out1 is emitted late (key not ready yet)

# misc fp32 column layout: [sb(2) tb(2) f2b(2) f3b128(1)]
MISC_W = 7

_cache = {}


def build_nc():
    import concourse.bass as bass
    import concourse.tile as tile
    from concourse import bacc, mybir
    from contextlib import ExitStack

    fp32 = mybir.dt.float32
    fp16 = mybir.dt.float16
    AF = mybir.ActivationFunctionType
    ALU = mybir.AluOpType

    nc = bacc.Bacc("TRN2")

    # ---- I/O (multi-chunk tensors prepacked chunk-major on host, fp16) ----
    xTm = nc.dram_tensor("xTm", [128, HC * L], fp16, kind="ExternalInput")
    tWTm = nc.dram_tensor("tWTm", [128, HC * E], fp16, kind="ExternalInput")
    xTim = nc.dram_tensor("xTim", [128, HC * IB], fp16, kind="ExternalInput")
    sWTm = nc.dram_tensor("sWTm", [128, HC * E], fp16, kind="ExternalInput")
    WuTm = nc.dram_tensor("WuTm", [128, EC * E], fp16, kind="ExternalInput")
    WvTm = nc.dram_tensor("WvTm", [128, EC * E], fp16, kind="ExternalInput")
    blWTm = nc.dram_tensor("blWTm", [128, EC * E * N], fp16, kind="ExternalInput")
    f3padm = nc.dram_tensor("f3padm", [128, EC * 48], fp16, kind="ExternalInput")
    misc = nc.dram_tensor("misc", [128, MISC_W], fp32, kind="ExternalInput")
    # raw pair-major layout: [pair, psum row (32k+12s+n, pads included), o, j];
    # host-side _gather unscrambles (and drops the 8 pad rows per 32-group)
    out = nc.dram_tensor("out", [PAIRS, 128, 2 * L], fp16, kind="ExternalOutput")

    with tile.TileContext(nc) as tc, ExitStack() as ctx:
        consts = ctx.enter_context(tc.tile_pool(name="consts", bufs=1))
        acts = ctx.enter_context(tc.tile_pool(name="acts", bufs=1))

        # Per-queue DMA bandwidth is only ~80 GB/s, so every large tensor is
        # split into chunk-DMAs spread over the three queues (sync/scalar/
        # gpsimd), ordered first-needed-first per queue.
        def tile_of(shape, name, dt=fp16):
            return consts.tile(shape, dt, name=name)

        def load_split(dst_t, src, width, engs):
            # dst_t/src are [128, W]; split into len(engs) column ranges
            nch = len(engs)
            step = width // nch
            for c, eng in enumerate(engs):
                lo, hi = step * c, (step * (c + 1) if c < nch - 1 else width)
                eng.dma_start(out=dst_t[:, lo:hi], in_=src[:, lo:hi])

        xT_m = tile_of([128, HC * L], "xT_m")
        tWT_m = tile_of([128, HC * E], "tWT_m")
        xTi_m = tile_of([128, HC * IB], "xTi_m")
        sWT_m = tile_of([128, HC * E], "sWT_m")
        misc_sb = tile_of([128, MISC_W], "misc_sb", dt=fp32)
        WuT_m = tile_of([128, EC * E], "WuT_m")
        WvT_m = tile_of([128, EC * E], "WvT_m")
        blWT_m = tile_of([128, EC * E * N], "blWT_m")
        f3pad_m = tile_of([128, EC * 48], "f3pad_m")

        # s-path inputs first (sT matmuls run while the bigger x load streams),
        # then x/tW interleaved per chunk-need order, then f2/bl weights.
        nc.scalar.dma_start(out=xTi_m[:], in_=xTim[:])
        nc.gpsimd.dma_start(out=sWT_m[:], in_=sWTm[:])
        nc.scalar.dma_start(out=misc_sb[:], in_=misc[:])
        nc.scalar.dma_start(out=WuT_m[:], in_=WuTm[:])
        load_split(xT_m, xTm[:], HC * L, [nc.sync, nc.scalar, nc.gpsimd])
        load_split(tWT_m, tWTm[:], HC * E, [nc.sync, nc.scalar])
        nc.sync.dma_start(out=WvT_m[:], in_=WvTm[:])
        nc.gpsimd.dma_start(out=f3pad_m[:], in_=f3padm[:])
        load_split(blWT_m, blWTm[:], EC * E * N,
                   [nc.sync, nc.scalar, nc.gpsimd])

        xT_sb = [xT_m[:, L * c : L * (c + 1)] for c in range(HC)]
        tWT_sb = [tWT_m[:, E * c : E * (c + 1)] for c in range(HC)]
        xTi_sb = [xTi_m[:, IB * c : IB * (c + 1)] for c in range(HC)]
        sWT_sb = [sWT_m[:, E * c : E * (c + 1)] for c in range(HC)]
        WuT_sb = [WuT_m[:, E * c : E * (c + 1)] for c in range(EC)]
        WvT_sb = [WvT_m[:, E * c : E * (c + 1)] for c in range(EC)]
        blWT3 = [blWT_m[:, E * N * c : E * N * (c + 1)]
                 .rearrange("p (e n) -> p e n", n=N) for c in range(EC)]
        f3pad_sb = [f3pad_m[:, 48 * c : 48 * (c + 1)] for c in range(EC)]
        o_ = 0
        sb_sb = misc_sb[:, o_ : o_ + 2]; o_ += 2
        tb_sb = misc_sb[:, o_ : o_ + 2]; o_ += 2
        f2b_sb = misc_sb[:, o_ : o_ + 2]; o_ += 2
        f3b_sb = misc_sb[:, o_ : o_ + 1]; o_ += 1

        # ---- persistent activations ----
        tT_sb = [acts.tile([128, L], fp16, name=f"tT{ec}") for ec in range(EC)]
        sTb_sb = [acts.tile([128, IB], fp16, name=f"sTb{ec}") for ec in range(EC)]
        tvT2c = acts.tile([128, 2 * L], fp16, name="tvT2c")  # cols 512*ec + j
        suT = acts.tile([128, 2 * IB], fp32, name="suT")  # cols IB*ec + i
        # keyNZ[ec][e, 128*(12s+n) + i] = key[i, 128ec+e, n] if i%2==s else 0
        keyNZ = [acts.tile([128, 24 * IB], fp16, name=f"keyNZ{ec}")
                 for ec in range(EC)]
        for ec in range(EC):
            # DVE is idle during the load phase; keep the gpsimd DMA queue free
            nc.vector.memset(keyNZ[ec][:], 0.0)

        # ---- prep A: s/su first (their inputs land early), then t/tv ----
        with tc.tile_pool(name="prepA", bufs=3, space="PSUM") as ppA:
            for ec in range(EC):
                ps_s = ppA.tile([128, L], fp32, name="ps_s", tag="ps")
                for hc in range(HC):
                    nc.tensor.matmul(
                        ps_s[:, :IB],
                        lhsT=sWT_sb[hc][:, 128 * ec : 128 * (ec + 1)],
                        rhs=xTi_sb[hc],
                        start=(hc == 0),
                        stop=(hc == HC - 1),
                    )
                nc.scalar.activation(sTb_sb[ec][:], ps_s[:, :IB], AF.Relu,
                                     bias=sb_sb[:, ec : ec + 1])

            for ec in range(EC):
                ps_su = ppA.tile([128, L], fp32, name="ps_su", tag="ps")
                for epc in range(EC):
                    nc.tensor.matmul(
                        ps_su[:, :IB],
                        lhsT=WuT_sb[epc][:, 128 * ec : 128 * (ec + 1)],
                        rhs=sTb_sb[epc][:],
                        start=(epc == 0),
                        stop=(epc == EC - 1),
                    )
                nc.scalar.activation(suT[:, IB * ec : IB * (ec + 1)],
                                     ps_su[:, :IB], AF.Identity,
                                     bias=f2b_sb[:, ec : ec + 1])

            for ec in range(EC):
                ps_t = ppA.tile([128, L], fp32, name="ps_t", tag="ps")
                for hc in range(HC):
                    nc.tensor.matmul(
                        ps_t[:],
                        lhsT=tWT_sb[hc][:, 128 * ec : 128 * (ec + 1)],
                        rhs=xT_sb[hc],
                        start=(hc == 0),
                        stop=(hc == HC - 1),
                    )
                nc.scalar.activation(tT_sb[ec][:], ps_t[:], AF.Relu,
                                     bias=tb_sb[:, ec : ec + 1])

            for ec in range(EC):
                ps_tv = ppA.tile([128, L], fp32, name="ps_tv", tag="ps")
                for epc in range(EC):
                    nc.tensor.matmul(
                        ps_tv[:],
                        lhsT=WvT_sb[epc][:, 128 * ec : 128 * (ec + 1)],
                        rhs=tT_sb[epc][:],
                        start=(epc == 0),
                        stop=(epc == EC - 1),
                    )
                nc.vector.tensor_copy(out=tvT2c[:, L * ec : L * (ec + 1)],
                                      in_=ps_tv[:])

        # ---- prep B: key, n-major psum (contiguous in-bank writes), then
        # interleave-casts into the zero-padded keyNZ layout.  ec=1 is
        # emitted mid-main-loop so the PE isn't head-of-line blocked on the
        # blW load.
        ppB = ctx.enter_context(tc.tile_pool(name="prepB", bufs=1, space="PSUM"))

        def emit_key(ec):
            psK = ppB.tile([128, N * IB], fp32, name="psK", tag="psK")
            for n in range(N):
                for epc in range(EC):
                    nc.tensor.matmul(
                        psK[:, IB * n : IB * (n + 1)],
                        lhsT=blWT3[epc][:, 128 * ec : 128 * (ec + 1), n],
                        rhs=sTb_sb[epc][:],
                        start=(epc == 0),
                        stop=(epc == EC - 1),
                    )
            psK3 = psK.rearrange("p (n i) -> p n i", i=IB)
            kz3_ = keyNZ[ec].rearrange("p (m i) -> p m i", i=IB)
            for s in range(2):
                src = psK3[:, :, s : IB : 2]
                dst = kz3_[:, N * s : N * (s + 1), s : IB : 2]
                if s == 0:
                    nc.vector.tensor_copy(out=dst, in_=src)
                else:
                    nc.scalar.copy(dst, src)

        emit_key(0)

        # ---- main loop: pairs of octets; flush pipelined one pair back ----
        hp = ctx.enter_context(tc.tile_pool(name="hp", bufs=44))
        outp = ctx.enter_context(tc.tile_pool(name="outp", bufs=3))
        mp = ctx.enter_context(tc.tile_pool(name="mp", bufs=2, space="PSUM"))

        kz3 = [keyNZ[ec].rearrange("p (m i) -> p m i", i=IB) for ec in range(EC)]

        def emit_out1(ps, o, first, last=False):
            # k-inner so consecutive matmuls land on different PE col-groups
            # (LDWEIGHTS of one group overlaps the stream of another)
            base = 512 * (o % 2)
            for ec in range(EC):
                for s in range(2):
                    for k in range(4):
                        i = 8 * o + 2 * k + s
                        nc.tensor.matmul(
                            ps[32 * k : 32 * k + 24, base : base + 512],
                            lhsT=kz3[ec][:, :, i],
                            rhs=tT_sb[ec][:],
                            start=(first and ec == 0 and s == 0),
                            stop=(last and ec == EC - 1 and s == 1),
                            tile_position=(0, 32 * k),
                            skip_group_check=True,
                        )

        def emit_h(o):
            hs = {}
            for p_ in range(8):
                i = 8 * o + p_
                for ec in range(EC):
                    ht = hp.tile([128, L], fp16, name="ht", tag="h")
                    if p_ < 6:
                        nc.vector.tensor_scalar(
                            out=ht[:], in0=tvT2c[:, L * ec : L * (ec + 1)],
                            scalar1=suT[:, IB * ec + i : IB * ec + i + 1],
                            scalar2=0.0, op0=ALU.add, op1=ALU.max)
                    else:
                        nc.scalar.activation(
                            ht[:], tvT2c[:, L * ec : L * (ec + 1)], AF.Relu,
                            bias=suT[:, IB * ec + i : IB * ec + i + 1])
                    hs[(p_, ec)] = ht
            return hs

        def emit_out2(ps, o, hs, first, last=True):
            base = 512 * (o % 2)
            for ec in range(EC):
                for p_ in (0, 2, 4, 6, 1, 3, 5, 7):
                    k, s = divmod(p_, 2)
                    nc.tensor.matmul(
                        ps[32 * k : 32 * k + 24, base : base + 512],
                        lhsT=f3pad_sb[ec][:, 24 * s : 24 * s + 24],
                        rhs=hs[(p_, ec)][:],
                        start=(first and ec == 0 and s == 0 and p_ == 2 * k),
                        stop=(last and ec == EC - 1),
                        tile_position=(0, 32 * k),
                        skip_group_check=True,
                    )

        def flush(pending):
            ps_prev, g_prev = pending
            ob = outp.tile([128, 2 * L], fp16, name="ob")
            nc.scalar.activation(ob[:], ps_prev[:], AF.Identity, bias=f3b_sb)
            # whole tile incl. pad rows -> one cheap fully-contiguous DMA
            nc.sync.dma_start(out=out[g_prev], in_=ob[:])

        # flush runs 2 pairs behind, emitted at loop top so ACT reaches it
        # long after the pair's matmuls completed (no head-of-line stall)
        pending = []  # [(psum pair tile, pair idx), ...]
        deferred = []  # (ps, o) waiting for key
        for g in range(PAIRS):
            if len(pending) >= 2:
                flush(pending.pop(0))
            ps = mp.tile([128, 2 * L], fp32, name="ps")
            for oo in range(2):
                o = 2 * g + oo
                hs = emit_h(o)
                if o < DEFER:
                    emit_out2(ps, o, hs, first=True, last=False)
                    deferred.append((ps, o))
                else:
                    emit_out1(ps, o, first=True)
                    emit_out2(ps, o, hs, first=False)
                if o == 0:
                    emit_key(1)
                if o == DEFER - 1:
                    for dps, do in deferred:
                        emit_out1(dps, do, first=False, last=True)
                    deferred = []
            pending.append((ps, g))
        for dps, do in deferred:  # DEFER > OCTS edge case
            emit_out1(dps, do, first=False, last=True)
        for pnd in pending:
            flush(pnd)

    nc.compile()
    return nc


def _get_nc():
    if "nc" not in _cache:
        _cache["nc"] = build_nc()
    return _cache["nc"]


def _chunk_major(a, nchunks):
    # [128*nchunks, W] -> [128, nchunks*W] with chunk-major free layout
    W = a.shape[1]
    return np.ascontiguousarray(
        a.reshape(nchunks, 128, W).transpose(1, 0, 2).reshape(128, nchunks * W))


def _make_in_maps(inputs):
    x = np.asarray(inputs["x"], np.float32)
    f32 = lambda a: np.asarray(a, np.float32)
    f16 = np.float16

    f2W = f32(inputs["f2W"])
    f3WT = f32(inputs["f3W"]).T  # [E, N]
    f3pad = np.zeros((E, 48), np.float32)
    for s in range(2):
        f3pad[:, 24 * s + 12 * s : 24 * s + 12 * s + N] = f3WT

    misc = np.zeros((128, MISC_W), np.float32)
    o_ = 0
    misc[:, o_ : o_ + 2] = f32(inputs["sb"]).reshape(EC, 128).T; o_ += 2
    misc[:, o_ : o_ + 2] = f32(inputs["tb"]).reshape(EC, 128).T; o_ += 2
    misc[:, o_ : o_ + 2] = f32(inputs["f2b"]).reshape(EC, 128).T; o_ += 2
    for k in range(4):
        for s in range(2):
            misc[32 * k + 12 * s : 32 * k + 12 * s + N, o_] = f32(inputs["f3b"])
    o_ += 1

    shared = {
        "sWTm": _chunk_major(f32(inputs["sW"]).T, HC).astype(f16),
        "tWTm": _chunk_major(f32(inputs["tW"]).T, HC).astype(f16),
        "WuTm": _chunk_major(f2W[:, :E].T, EC).astype(f16),
        "WvTm": _chunk_major(f2W[:, E:].T, EC).astype(f16),
        "blWTm": _chunk_major(f32(inputs["blW"]).T, EC).astype(f16),
        "f3padm": _chunk_major(f3pad, EC).astype(f16),
        "misc": misc,
    }

    in_maps = []
    for c in range(NCORES):
        b, r = divmod(c, 4)
        m = dict(shared)
        m["xTm"] = _chunk_major(np.ascontiguousarray(x[b].T), HC).astype(f16)
        m["xTim"] = _chunk_major(
            np.ascontiguousarray(x[b, IB * r : IB * (r + 1), :].T), HC).astype(f16)
        in_maps.append(m)
    return in_maps


def _gather(results):
    full = np.empty((B, L, N, L), np.float32)
    for c in range(NCORES):
        b, r = divmod(c, 4)
        raw = results[c]["out"]  # [PAIRS, 128, 2L] fp16
        # row 32k + 12s + n of pair g, col 512*o + j  ->  (i = 16g+8o+2k+s, n, j)
        v = raw.reshape(PAIRS, 4, 32, 2, L)[:, :, :24]  # drop pads
        v = v.reshape(PAIRS, 4, 2, N, 2, L)  # [g, k, s, n, o, j]
        v = v.transpose(0, 4, 1, 2, 3, 5)  # [g, o, k, s, n, j]
        full[b, IB * r : IB * (r + 1)] = v.reshape(IB, N, L)
    return full


def kernel(x, sW, sb, tW, tb, f2W, f2b, f3W, f3b, blW):
    from concourse.bass_utils import run_bass_kernel_spmd

    in_maps = _make_in_maps(dict(
        x=x, sW=sW, sb=sb, tW=tW, tb=tb, f2W=f2W, f2b=f2b,
        f3W=f3W, f3b=f3b, blW=blW,
    ))
    nc = _get_nc()
    res = run_bass_kernel_spmd(nc, in_maps, core_ids=list(range(NCORES)))
    return _gather(res.results)
